# revision 17
# baseline (speedup 1.0000x reference)
"""Trainium2 Bass kernel for nn_Attention2 (attention-gated blend of Z_l/Z_g).

Reference math:
    Q     = Z_o @ W.T + b                      # [N, 512]
    att_l = Q @ colsum(Z_l)                    # [N]
    att_g = Q @ colsum(Z_g)                    # [N]
    att   = softmax([att_l, att_g], axis=1)    # [N, 2]
    out   = Z_l * att[:, 0:1] + Z_g * att[:, 1:2]

Only d = att_l - att_g matters (2-way softmax == sigmoid), and it folds:
    s = colsum(Z_l) - colsum(Z_g)              # [512]
    u = W.T @ s                                # [512]
    c = b . s                                  # scalar
    d = Z_o @ u + c                            # [N]
    out = Z_g + sigmoid(d) * (Z_l - Z_g)

Fused single-launch SPMD design (8 cores, rows sharded, fp16 I/O).

Row packing: each 512-row chunk maps partition q <- rows r0+4q..r0+4q+3
(4KB contiguous per partition = 1 DMA descriptor; DMA issue cost scales with
descriptor count). The shard tail is 212 rows = 4 x 53, so every chunk is
uniform with Q = 128 or 53 partitions. Z_o is uploaded transposed with
columns permuted per chunk so position s*Q + q <-> row r0+4q+s and stage-2
matmul weights slice contiguously.

  Stage 1: stream Z_l (SyncE issue) / Z_g (ScalarE issue) one chunk per DMA;
    colsum via +-1 ones matmuls into PSUM (4 col-slices per chunk); cache
    zd = Z_l - Z_g (all chunks, one DVE sub per chunk) and Z_g (most chunks).
  AllReduce(add) of s_partial [1,512] f32 (DRAM bounce); the first 3 ZoT
    chunk loads are emitted right after stage 1 so their transfers fill the
    collective bubble.
  Interlude: u = W.T @ s (W preloaded in one merged DMA during stage 1);
    u split into fp16 (hi, lo) pair to keep f32 accuracy; c = b.s broadcast.
  Stage 2 per chunk: 32 matmuls (4 s-slices x 4 k-chunks x hi/lo) accumulate
    d in PSUM [Q,1] per s-slice; p = sigmoid(d + c) on ScalarE (reads PSUM);
    out-slice = (zd * p) + zg in one DVE scalar_tensor_tensor; one merged
    out DMA per chunk (SyncE issue).

Precision: fp16 inputs give rel err ~8e-3 vs the f32 reference (gate 2e-2).
"""

import numpy as np

import concourse.bacc as bacc
import concourse.mybir as mybir
import concourse.tile as tile
from concourse.bass_types import AP
from concourse.bass_utils import run_bass_kernel_spmd

N_CORES = 8
N_TOTAL = 100000
CH = 512
SHARD = N_TOTAL // N_CORES  # 12500
P = 128
CHUNK = 4 * P  # 512 rows per chunk
N_CHUNKS = (SHARD + CHUNK - 1) // CHUNK  # 25; last chunk 212 rows (53 x 4)
C_ZG = 12  # Z_g chunks cached in SBUF; rest re-read in stage 2
ZOT_PRE = 3  # zot ring depth / prefetch distance

f16 = mybir.dt.float16
f32 = mybir.dt.float32


def _chunk_rows_ap(dram_t, r0, q):
    """[q, 4*CH] view: partition j <- DRAM rows r0+4j..r0+4j+3 (contiguous)."""
    h = dram_t[0:1].tensor
    return AP(h, r0 * CH, [[4 * CH, q], [1, 4 * CH]])


def _zot_ap(dram_t, c0, gw):
    """[P, 4, gw] view: partition p, seg k, col r -> zoT[k*P+p, c0+r]."""
    h = dram_t[0:1].tensor
    return AP(h, c0, [[SHARD, P], [P * SHARD, 4], [1, gw]])


def _emit_body(nc, pools, tensors, c_zg, no_cc=False, ring3=False):
    add = mybir.AluOpType.add
    mult = mybir.AluOpType.mult
    AF = mybir.ActivationFunctionType
    fix, s1, zop, otp, smp, psfix, psd, dram = pools
    (zl_d, zg_d, zoT_d, w_d, b_d, out_d, consts, zd_cache, zg_cache) = tensors
    ones, nones, one11_16, ones_row, one11_32 = consts

    def chunk_q(ci):
        return min(P, (SHARD - ci * CHUNK + 3) // 4)

    # ---------------- Stage 1: colsum + cache fill ----------------
    # W preload (no deps; lands during stage 1)
    wq = fix.tile([P, 4 * CH], f32, tag="wq")
    h_w = w_d[0:1].tensor
    nc.sync.dma_start(wq[:, :], AP(h_w, 0, [[CH, P], [P * CH, 4], [1, CH]]))
    ps_s = psfix.tile([1, CH], f32, tag="ps_s")
    for ci in range(N_CHUNKS):
        r0 = ci * CHUNK
        Q = chunk_q(ci)
        zl4 = s1.tile([P, 4 * CH], f16, tag="zl4", bufs=2)
        nc.sync.dma_start(zl4[:Q, :], _chunk_rows_ap(zl_d, r0, Q))
        if ci < c_zg:
            zg4, zgc0 = zg_cache, ci * 4 * CH
        else:
            zg4 = s1.tile([P, 4 * CH], f16, tag="zgr", bufs=2, name="zg4t")
            zgc0 = 0
        zg_eng = nc.gpsimd if ring3 else nc.scalar
        zg_eng.dma_start(
            zg4[:Q, zgc0 : zgc0 + 4 * CH], _chunk_rows_ap(zg_d, r0, Q)
        )
        for s in range(4):
            nc.tensor.matmul(
                ps_s[:], ones[:Q], zl4[:Q, s * CH : (s + 1) * CH],
                start=(ci == 0 and s == 0), stop=False,
            )
            nc.tensor.matmul(
                ps_s[:], nones[:Q], zg4[:Q, zgc0 + s * CH : zgc0 + (s + 1) * CH],
                start=False, stop=(ci == N_CHUNKS - 1 and s == 3),
            )
        nc.vector.tensor_sub(
            zd_cache[:Q, ci * 4 * CH : (ci + 1) * 4 * CH],
            zl4[:Q, :],
            zg4[:Q, zgc0 : zgc0 + 4 * CH],
        )

    # ---------------- AllReduce of s ----------------
    s_sb = fix.tile([1, CH], f32, tag="s_sb")
    nc.vector.tensor_copy(s_sb[:], ps_s[:])
    s_part = dram.tile([1, CH], f32, tag="s_part", bufs=2)
    s_glob = dram.tile([1, CH], f32, tag="s_glob", bufs=2)
    nc.sync.dma_start(s_part[:, :], s_sb[:])
    # zot prefetch for the first chunks: no dep on the collective, so these
    # transfers fill the collective bubble.
    zots = {}
    for ci in range(min(ZOT_PRE, N_CHUNKS)):
        zt = zop.tile([P, 4 * CHUNK], f16, tag="zot", bufs=ZOT_PRE, name="zott")
        gw = chunk_q(ci) * 4
        nc.scalar.dma_start(zt[:, : 4 * gw], _zot_ap(zoT_d, ci * CHUNK, gw))
        zots[ci] = zt
    if no_cc:
        nc.sync.dma_start(s_glob[:, :], s_part[:, :])
    else:
        nc.gpsimd.collective_compute(
            "AllReduce",
            add,
            replica_groups=[list(range(N_CORES))],
            ins=[s_part.opt()],
            outs=[s_glob.opt()],
        )
    s_all = fix.tile([1, CH], f32, tag="s_all")
    nc.sync.dma_start(s_all[:], s_glob[:, :])

    # ---------------- Interlude: u, c on device ----------------
    ps_u = psfix.tile([1, CH], f32, tag="ps_u")
    ps_c = psfix.tile([1, 1], f32, tag="ps_c")
    scks = []
    for k in range(4):
        ps_sc = psd.tile([P, 1], f32, tag="tr", bufs=2)
        nc.tensor.matmul(
            ps_sc[:], s_all[0:1, k * P : (k + 1) * P], one11_32[:],
            start=True, stop=True,
        )
        sck = fix.tile([P, 1], f32, tag=f"sck{k}")
        nc.vector.tensor_copy(sck[:], ps_sc[:])
        scks.append(sck)
    for k in range(4):
        nc.tensor.matmul(
            ps_u[:], scks[k][:], wq[:, k * CH : (k + 1) * CH],
            start=(k == 0), stop=(k == 3),
        )
    for k in range(4):
        bk = fix.tile([P, 1], f32, tag=f"bk{k}")
        nc.sync.dma_start(bk[:], b_d[k * P : (k + 1) * P, 0:1])
        nc.tensor.matmul(ps_c[:], scks[k][:], bk[:], start=(k == 0), stop=(k == 3))
    c_sb = fix.tile([1, 1], f32, tag="c_sb")
    nc.vector.tensor_copy(c_sb[:], ps_c[:])
    ps_cb = psd.tile([P, 1], f32, tag="tr", bufs=2)
    nc.tensor.matmul(ps_cb[:], ones_row[:], c_sb[:], start=True, stop=True)
    c_b = fix.tile([P, 1], f32, tag="c_b")
    nc.vector.tensor_copy(c_b[:], ps_cb[:])

    u_hi = fix.tile([1, CH], f16, tag="u_hi")
    nc.vector.tensor_copy(u_hi[:], ps_u[:])
    u_hi32 = fix.tile([1, CH], f32, tag="u_hi32")
    nc.vector.tensor_copy(u_hi32[:], u_hi[:])
    u_lo = fix.tile([1, CH], f16, tag="u_lo")
    nc.vector.tensor_sub(u_lo[:], ps_u[:], u_hi32[:])
    u2 = []
    for k in range(4):
        u2k = fix.tile([P, 2], f16, tag=f"u2_{k}")
        for h, src in enumerate((u_hi, u_lo)):
            ps_tr = psd.tile([P, 1], f32, tag="tr", bufs=2)
            nc.tensor.matmul(
                ps_tr[:], src[0:1, k * P : (k + 1) * P], one11_16[:],
                start=True, stop=True,
            )
            nc.vector.tensor_copy(u2k[:, h : h + 1], ps_tr[:])
        u2.append(u2k)

    # ---------------- Stage 2: d, sigmoid, blend ----------------
    for ci in range(N_CHUNKS):
        r0 = ci * CHUNK
        Q = chunk_q(ci)
        GW = 4 * Q
        zot = zots.pop(ci)
        pre = ci + ZOT_PRE
        if pre < N_CHUNKS:
            zt = zop.tile([P, 4 * CHUNK], f16, tag="zot", bufs=ZOT_PRE, name="zotl")
            gwp = chunk_q(pre) * 4
            nc.scalar.dma_start(zt[:, : 4 * gwp], _zot_ap(zoT_d, pre * CHUNK, gwp))
            zots[pre] = zt
        if ci < c_zg:
            zgb, zgc0 = zg_cache, ci * 4 * CH
        else:
            zgb = s1.tile([P, 4 * CH], f16, tag="zgr", bufs=2, name="zgrt")
            zgc0 = 0
            (nc.gpsimd if ring3 else nc.sync).dma_start(
                zgb[:Q, :], _chunk_rows_ap(zg_d, r0, Q)
            )
        outm = otp.tile([P, 4 * CH], f16, tag="outm", bufs=2)
        for s in range(4):
            ps_d = psd.tile([P, 2], f32, tag="d", bufs=3)
            for k in range(4):
                lhs = zot[:, k * GW + s * Q : k * GW + (s + 1) * Q]
                nc.tensor.matmul(
                    ps_d[:Q], lhs, u2[k][:, 0:2], start=(k == 0), stop=(k == 3)
                )
            d_sb = smp.tile([P, 1], f32, tag="dsb")
            nc.vector.tensor_reduce(d_sb[:Q], ps_d[:Q, 0:2], axis=mybir.AxisListType.X, op=add)
            p_t = smp.tile([P, 1], f32, tag="p")
            nc.scalar.activation(
                p_t[:Q], d_sb[:Q], AF.Sigmoid, bias=c_b[:Q, 0:1], scale=1.0
            )
            nc.vector.scalar_tensor_tensor(
                outm[:Q, s * CH : (s + 1) * CH],
                zd_cache[:Q, ci * 4 * CH + s * CH : ci * 4 * CH + (s + 1) * CH],
                p_t[:Q, 0:1],
                zgb[:Q, zgc0 + s * CH : zgc0 + (s + 1) * CH],
                op0=mult,
                op1=add,
            )
        out_eng = (nc.scalar if (ring3 and ci % 2) else nc.sync)
        out_eng.dma_start(_chunk_rows_ap(out_d, r0, Q), outm[:Q, :])


def build_nc(c_zg=C_ZG, bufs=4, rep_loop=1, rep_mode="unroll", no_cc=False, ring3=False):
    import contextlib

    nc = bacc.Bacc(
        "TRN2",
        target_bir_lowering=False,
        debug=False,
        enable_asserts=False,
        num_devices=N_CORES,
    )
    zl_d = nc.dram_tensor("Z_l", [SHARD, CH], f16, kind="ExternalInput")
    zg_d = nc.dram_tensor("Z_g", [SHARD, CH], f16, kind="ExternalInput")
    zoT_d = nc.dram_tensor("ZoT", [CH, SHARD], f16, kind="ExternalInput")
    w_d = nc.dram_tensor("W", [CH, CH], f32, kind="ExternalInput")
    b_d = nc.dram_tensor("b", [CH, 1], f32, kind="ExternalInput")
    out_d = nc.dram_tensor("out", [SHARD, CH], f16, kind="ExternalOutput")

    with tile.TileContext(nc) as tc:
        with (
            tc.tile_pool(name="cache", bufs=1) as cache,
            tc.tile_pool(name="fix", bufs=1) as fix,
            tc.tile_pool(name="s1", bufs=2) as s1,
            tc.tile_pool(name="zo", bufs=2) as zop,
            tc.tile_pool(name="ot", bufs=2) as otp,
            tc.tile_pool(name="sm", bufs=4) as smp,
            tc.tile_pool(name="psfix", bufs=1, space="PSUM") as psfix,
            tc.tile_pool(name="psd", bufs=4, space="PSUM") as psd,
            tc.tile_pool(name="dram", bufs=1, space="DRAM") as dram,
        ):
            ones = fix.tile([P, 1], f16, tag="ones")
            nones = fix.tile([P, 1], f16, tag="nones")
            one11_16 = fix.tile([1, 1], f16, tag="one11_16")
            ones_row = fix.tile([1, P], f32, tag="ones_row")
            one11_32 = fix.tile([1, 1], f32, tag="one11_32")
            nc.vector.memset(ones[:], 1.0)
            nc.vector.memset(nones[:], -1.0)
            nc.vector.memset(one11_16[:], 1.0)
            nc.vector.memset(ones_row[:], 1.0)
            nc.vector.memset(one11_32[:], 1.0)
            consts = (ones, nones, one11_16, ones_row, one11_32)

            zd_cache = cache.tile([P, N_CHUNKS * 4 * CH], f16, tag="zd")
            zg_cache = cache.tile([P, c_zg * 4 * CH], f16, tag="zg")

            pools = (fix, s1, zop, otp, smp, psfix, psd, dram)
            tensors = (
                zl_d, zg_d, zoT_d, w_d, b_d, out_d, consts, zd_cache, zg_cache
            )
            if rep_loop > 1 and rep_mode == "unroll":
                for _ in range(rep_loop):
                    _emit_body(nc, pools, tensors, c_zg, no_cc=no_cc, ring3=ring3)
            else:
                rep_ctx = (
                    tc.For_i(0, rep_loop, 1)
                    if rep_loop > 1
                    else contextlib.nullcontext()
                )
                with rep_ctx:
                    _emit_body(nc, pools, tensors, c_zg, no_cc=no_cc, ring3=ring3)
    nc.compile()
    return nc


_CACHE = {}


# column permutation applied to zoT within each 512-row chunk:
# position s*Q + q  <->  row r0 + 4q + s
def _zot_perm():
    idx = np.empty(SHARD, dtype=np.int64)
    for ci in range(N_CHUNKS):
        r0 = ci * CHUNK
        n = min(CHUNK, SHARD - r0)
        q = n // 4
        idx[r0 : r0 + n] = r0 + np.arange(n).reshape(q, 4).T.reshape(-1)
    return idx


_ZOT_IDX = _zot_perm()


def _prep_maps(Z_o, Z_l, Z_g, W, b):
    W32 = np.ascontiguousarray(np.asarray(W, dtype=np.float32))
    b32 = np.ascontiguousarray(np.asarray(b, dtype=np.float32).reshape(CH, 1))
    maps = []
    for i in range(N_CORES):
        sl = slice(i * SHARD, (i + 1) * SHARD)
        zo16 = np.asarray(Z_o[sl], dtype=np.float16)
        zoT = np.ascontiguousarray(zo16[_ZOT_IDX].T)
        maps.append(
            {
                "Z_l": np.ascontiguousarray(np.asarray(Z_l[sl], dtype=np.float16)),
                "Z_g": np.ascontiguousarray(np.asarray(Z_g[sl], dtype=np.float16)),
                "ZoT": zoT,
                "W": W32,
                "b": b32,
            }
        )
    return maps


def kernel(Z_o, Z_l, Z_g, W, b):
    if "nc" not in _CACHE:
        _CACHE["nc"] = build_nc()
    nc = _CACHE["nc"]
    maps = _prep_maps(Z_o, Z_l, Z_g, W, b)
    res = run_bass_kernel_spmd(nc, maps, core_ids=list(range(N_CORES)))
    out = np.concatenate([r["out"] for r in res.results], axis=0)
    return out.astype(np.float32)


# revision 18
# speedup vs baseline: 1.1265x; 1.1265x over previous
"""Trainium2 Bass kernel for nn_Attention2 (attention-gated blend of Z_l/Z_g).

Reference math:
    Q     = Z_o @ W.T + b                      # [N, 512]
    att_l = Q @ colsum(Z_l)                    # [N]
    att_g = Q @ colsum(Z_g)                    # [N]
    att   = softmax([att_l, att_g], axis=1)    # [N, 2]
    out   = Z_l * att[:, 0:1] + Z_g * att[:, 1:2]

Only d = att_l - att_g matters (2-way softmax == sigmoid), and it folds:
    s = colsum(Z_l) - colsum(Z_g)              # [512]
    u = W.T @ s                                # [512]
    c = b . s                                  # scalar
    d = Z_o @ u + c                            # [N]
    out = Z_g + sigmoid(d) * (Z_l - Z_g)

Fused single-launch SPMD design (8 cores, rows sharded, fp16 I/O).

Row packing: each 512-row chunk maps partition q <- rows r0+4q..r0+4q+3
(4KB contiguous per partition = 1 DMA descriptor; DMA issue cost scales with
descriptor count). The shard tail is 212 rows = 4 x 53, so every chunk is
uniform with Q = 128 or 53 partitions. Z_o is uploaded transposed with
columns permuted per chunk so position s*Q + q <-> row r0+4q+s and stage-2
matmul weights slice contiguously.

  Stage 1: stream Z_l (SyncE issue) / Z_g (ScalarE issue) one chunk per DMA;
    colsum via +-1 ones matmuls into PSUM (4 col-slices per chunk); cache
    zd = Z_l - Z_g (all chunks, one DVE sub per chunk) and Z_g (most chunks).
  AllReduce(add) of s_partial [1,512] f32 (DRAM bounce); the first 3 ZoT
    chunk loads are emitted right after stage 1 so their transfers fill the
    collective bubble.
  Interlude: u = W.T @ s (W preloaded in one merged DMA during stage 1);
    u split into fp16 (hi, lo) pair to keep f32 accuracy; c = b.s broadcast.
  Stage 2 per chunk: 32 matmuls (4 s-slices x 4 k-chunks x hi/lo) accumulate
    d in PSUM [Q,1] per s-slice; p = sigmoid(d + c) on ScalarE (reads PSUM);
    out-slice = (zd * p) + zg in one DVE scalar_tensor_tensor; one merged
    out DMA per chunk (SyncE issue).

Precision: fp16 inputs give rel err ~8e-3 vs the f32 reference (gate 2e-2).
"""

import numpy as np

import concourse.bacc as bacc
import concourse.mybir as mybir
import concourse.tile as tile
from concourse.bass_types import AP
from concourse.bass_utils import run_bass_kernel_spmd

N_CORES = 8
N_TOTAL = 100000
CH = 512
SHARD = N_TOTAL // N_CORES  # 12500
P = 128
CHUNK = 4 * P  # 512 rows per chunk
N_CHUNKS = (SHARD + CHUNK - 1) // CHUNK  # 25; last chunk 212 rows (53 x 4)
C_ZG = 12  # Z_g chunks cached in SBUF; rest re-read in stage 2
ZOT_PRE = 3  # zot ring depth / prefetch distance

f16 = mybir.dt.float16
f32 = mybir.dt.float32


def _chunk_rows_ap(dram_t, r0, q):
    """[q, 4*CH] view: partition j <- DRAM rows r0+4j..r0+4j+3 (contiguous)."""
    h = dram_t[0:1].tensor
    return AP(h, r0 * CH, [[4 * CH, q], [1, 4 * CH]])


def _zot_ap(dram_t, c0, gw):
    """[P, 4, gw] view: partition p, seg k, col r -> zoT[k*P+p, c0+r]."""
    h = dram_t[0:1].tensor
    return AP(h, c0, [[SHARD, P], [P * SHARD, 4], [1, gw]])


def _emit_body(nc, pools, tensors, c_zg, no_cc=False, ring3=False):
    add = mybir.AluOpType.add
    mult = mybir.AluOpType.mult
    AF = mybir.ActivationFunctionType
    fix, s1, zop, otp, smp, psfix, psd, dram = pools
    (zl_d, zg_d, zoT_d, w_d, b_d, out_d, consts, zd_cache, zg_cache) = tensors
    ones, nones, one11_16, ones_row, one11_32 = consts

    def chunk_q(ci):
        return min(P, (SHARD - ci * CHUNK + 3) // 4)

    # ---------------- Stage 1: colsum + cache fill ----------------
    # W preload (no deps; lands during stage 1)
    wq = fix.tile([P, 4 * CH], f32, tag="wq")
    h_w = w_d[0:1].tensor
    nc.sync.dma_start(wq[:, :], AP(h_w, 0, [[CH, P], [P * CH, 4], [1, CH]]))
    ps_s = psfix.tile([1, CH], f32, tag="ps_s")
    for ci in range(N_CHUNKS):
        r0 = ci * CHUNK
        Q = chunk_q(ci)
        zl4 = s1.tile([P, 4 * CH], f16, tag="zl4", bufs=2)
        nc.sync.dma_start(zl4[:Q, :], _chunk_rows_ap(zl_d, r0, Q))
        if ci < c_zg:
            zg4, zgc0 = zg_cache, ci * 4 * CH
        else:
            zg4 = s1.tile([P, 4 * CH], f16, tag="zgr", bufs=2, name="zg4t")
            zgc0 = 0
        zg_eng = nc.gpsimd if ring3 else nc.scalar
        zg_eng.dma_start(
            zg4[:Q, zgc0 : zgc0 + 4 * CH], _chunk_rows_ap(zg_d, r0, Q)
        )
        for s in range(4):
            nc.tensor.matmul(
                ps_s[:], ones[:Q], zl4[:Q, s * CH : (s + 1) * CH],
                start=(ci == 0 and s == 0), stop=False,
            )
            nc.tensor.matmul(
                ps_s[:], nones[:Q], zg4[:Q, zgc0 + s * CH : zgc0 + (s + 1) * CH],
                start=False, stop=(ci == N_CHUNKS - 1 and s == 3),
            )
        nc.vector.tensor_sub(
            zd_cache[:Q, ci * 4 * CH : (ci + 1) * 4 * CH],
            zl4[:Q, :],
            zg4[:Q, zgc0 : zgc0 + 4 * CH],
        )

    # ---------------- AllReduce of s ----------------
    s_sb = fix.tile([1, CH], f32, tag="s_sb")
    nc.vector.tensor_copy(s_sb[:], ps_s[:])
    s_part = dram.tile([1, CH], f32, tag="s_part", bufs=2)
    s_glob = dram.tile([1, CH], f32, tag="s_glob", bufs=2)
    nc.sync.dma_start(s_part[:, :], s_sb[:])
    # zot prefetch for the first chunks: no dep on the collective, so these
    # transfers fill the collective bubble.
    zots = {}
    for ci in range(min(ZOT_PRE, N_CHUNKS)):
        zt = zop.tile([P, 4 * CHUNK], f16, tag="zot", bufs=ZOT_PRE, name="zott")
        gw = chunk_q(ci) * 4
        nc.scalar.dma_start(zt[:, : 4 * gw], _zot_ap(zoT_d, ci * CHUNK, gw))
        zots[ci] = zt
    if no_cc:
        nc.sync.dma_start(s_glob[:, :], s_part[:, :])
    else:
        nc.gpsimd.collective_compute(
            "AllReduce",
            add,
            replica_groups=[list(range(N_CORES))],
            ins=[s_part.opt()],
            outs=[s_glob.opt()],
        )
    s_all = fix.tile([1, CH], f32, tag="s_all")
    nc.sync.dma_start(s_all[:], s_glob[:, :])

    # ---------------- Interlude: u, c on device ----------------
    ps_u = psfix.tile([1, CH], f32, tag="ps_u")
    ps_c = psfix.tile([1, 1], f32, tag="ps_c")
    scks = []
    for k in range(4):
        ps_sc = psd.tile([P, 1], f32, tag="tr", bufs=2)
        nc.tensor.matmul(
            ps_sc[:], s_all[0:1, k * P : (k + 1) * P], one11_32[:],
            start=True, stop=True,
        )
        sck = fix.tile([P, 1], f32, tag=f"sck{k}")
        nc.vector.tensor_copy(sck[:], ps_sc[:])
        scks.append(sck)
    for k in range(4):
        nc.tensor.matmul(
            ps_u[:], scks[k][:], wq[:, k * CH : (k + 1) * CH],
            start=(k == 0), stop=(k == 3),
        )
    for k in range(4):
        bk = fix.tile([P, 1], f32, tag=f"bk{k}")
        nc.sync.dma_start(bk[:], b_d[k * P : (k + 1) * P, 0:1])
        nc.tensor.matmul(ps_c[:], scks[k][:], bk[:], start=(k == 0), stop=(k == 3))
    c_sb = fix.tile([1, 1], f32, tag="c_sb")
    nc.vector.tensor_copy(c_sb[:], ps_c[:])
    ps_cb = psd.tile([P, 1], f32, tag="tr", bufs=2)
    nc.tensor.matmul(ps_cb[:], ones_row[:], c_sb[:], start=True, stop=True)
    c_b = fix.tile([P, 1], f32, tag="c_b")
    nc.vector.tensor_copy(c_b[:], ps_cb[:])

    u_hi = fix.tile([1, CH], f16, tag="u_hi")
    nc.vector.tensor_copy(u_hi[:], ps_u[:])
    u_hi32 = fix.tile([1, CH], f32, tag="u_hi32")
    nc.vector.tensor_copy(u_hi32[:], u_hi[:])
    u_lo = fix.tile([1, CH], f16, tag="u_lo")
    nc.vector.tensor_sub(u_lo[:], ps_u[:], u_hi32[:])
    u2 = []
    for k in range(4):
        u2k = fix.tile([P, 2], f16, tag=f"u2_{k}")
        for h, src in enumerate((u_hi, u_lo)):
            ps_tr = psd.tile([P, 1], f32, tag="tr", bufs=2)
            nc.tensor.matmul(
                ps_tr[:], src[0:1, k * P : (k + 1) * P], one11_16[:],
                start=True, stop=True,
            )
            nc.vector.tensor_copy(u2k[:, h : h + 1], ps_tr[:])
        u2.append(u2k)

    # ---------------- Stage 2: d, sigmoid, blend ----------------
    for ci in range(N_CHUNKS):
        r0 = ci * CHUNK
        Q = chunk_q(ci)
        GW = 4 * Q
        zot = zots.pop(ci)
        pre = ci + ZOT_PRE
        if pre < N_CHUNKS:
            zt = zop.tile([P, 4 * CHUNK], f16, tag="zot", bufs=ZOT_PRE, name="zotl")
            gwp = chunk_q(pre) * 4
            nc.scalar.dma_start(zt[:, : 4 * gwp], _zot_ap(zoT_d, pre * CHUNK, gwp))
            zots[pre] = zt
        if ci < c_zg:
            zgb, zgc0 = zg_cache, ci * 4 * CH
        else:
            zgb = s1.tile([P, 4 * CH], f16, tag="zgr", bufs=2, name="zgrt")
            zgc0 = 0
            (nc.gpsimd if ring3 else nc.sync).dma_start(
                zgb[:Q, :], _chunk_rows_ap(zg_d, r0, Q)
            )
        outm = otp.tile([P, 4 * CH], f16, tag="outm", bufs=2)
        for s in range(4):
            ps_d = psd.tile([P, 1], f32, tag="d", bufs=3)
            for k in range(4):
                lhs = zot[:, k * GW + s * Q : k * GW + (s + 1) * Q]
                nc.tensor.matmul(
                    ps_d[:Q], lhs, u2[k][:, 0:1], start=(k == 0), stop=False
                )
                nc.tensor.matmul(
                    ps_d[:Q], lhs, u2[k][:, 1:2], start=False, stop=(k == 3)
                )
            p_t = smp.tile([P, 1], f32, tag="p")
            nc.scalar.activation(
                p_t[:Q], ps_d[:Q], AF.Sigmoid, bias=c_b[:Q, 0:1], scale=1.0
            )
            nc.vector.scalar_tensor_tensor(
                outm[:Q, s * CH : (s + 1) * CH],
                zd_cache[:Q, ci * 4 * CH + s * CH : ci * 4 * CH + (s + 1) * CH],
                p_t[:Q, 0:1],
                zgb[:Q, zgc0 + s * CH : zgc0 + (s + 1) * CH],
                op0=mult,
                op1=add,
            )
        out_eng = (nc.scalar if (ring3 and ci % 2) else nc.sync)
        out_eng.dma_start(_chunk_rows_ap(out_d, r0, Q), outm[:Q, :])


def build_nc(c_zg=C_ZG, bufs=4, rep_loop=1, rep_mode="unroll", no_cc=False, ring3=False):
    import contextlib

    nc = bacc.Bacc(
        "TRN2",
        target_bir_lowering=False,
        debug=False,
        enable_asserts=False,
        num_devices=N_CORES,
    )
    zl_d = nc.dram_tensor("Z_l", [SHARD, CH], f16, kind="ExternalInput")
    zg_d = nc.dram_tensor("Z_g", [SHARD, CH], f16, kind="ExternalInput")
    zoT_d = nc.dram_tensor("ZoT", [CH, SHARD], f16, kind="ExternalInput")
    w_d = nc.dram_tensor("W", [CH, CH], f32, kind="ExternalInput")
    b_d = nc.dram_tensor("b", [CH, 1], f32, kind="ExternalInput")
    out_d = nc.dram_tensor("out", [SHARD, CH], f16, kind="ExternalOutput")

    with tile.TileContext(nc) as tc:
        with (
            tc.tile_pool(name="cache", bufs=1) as cache,
            tc.tile_pool(name="fix", bufs=1) as fix,
            tc.tile_pool(name="s1", bufs=2) as s1,
            tc.tile_pool(name="zo", bufs=2) as zop,
            tc.tile_pool(name="ot", bufs=2) as otp,
            tc.tile_pool(name="sm", bufs=4) as smp,
            tc.tile_pool(name="psfix", bufs=1, space="PSUM") as psfix,
            tc.tile_pool(name="psd", bufs=4, space="PSUM") as psd,
            tc.tile_pool(name="dram", bufs=1, space="DRAM") as dram,
        ):
            ones = fix.tile([P, 1], f16, tag="ones")
            nones = fix.tile([P, 1], f16, tag="nones")
            one11_16 = fix.tile([1, 1], f16, tag="one11_16")
            ones_row = fix.tile([1, P], f32, tag="ones_row")
            one11_32 = fix.tile([1, 1], f32, tag="one11_32")
            nc.vector.memset(ones[:], 1.0)
            nc.vector.memset(nones[:], -1.0)
            nc.vector.memset(one11_16[:], 1.0)
            nc.vector.memset(ones_row[:], 1.0)
            nc.vector.memset(one11_32[:], 1.0)
            consts = (ones, nones, one11_16, ones_row, one11_32)

            zd_cache = cache.tile([P, N_CHUNKS * 4 * CH], f16, tag="zd")
            zg_cache = cache.tile([P, c_zg * 4 * CH], f16, tag="zg")

            pools = (fix, s1, zop, otp, smp, psfix, psd, dram)
            tensors = (
                zl_d, zg_d, zoT_d, w_d, b_d, out_d, consts, zd_cache, zg_cache
            )
            if rep_loop > 1 and rep_mode == "unroll":
                for _ in range(rep_loop):
                    _emit_body(nc, pools, tensors, c_zg, no_cc=no_cc, ring3=ring3)
            else:
                rep_ctx = (
                    tc.For_i(0, rep_loop, 1)
                    if rep_loop > 1
                    else contextlib.nullcontext()
                )
                with rep_ctx:
                    _emit_body(nc, pools, tensors, c_zg, no_cc=no_cc, ring3=ring3)
    nc.compile()
    return nc


_CACHE = {}


# column permutation applied to zoT within each 512-row chunk:
# position s*Q + q  <->  row r0 + 4q + s
def _zot_perm():
    idx = np.empty(SHARD, dtype=np.int64)
    for ci in range(N_CHUNKS):
        r0 = ci * CHUNK
        n = min(CHUNK, SHARD - r0)
        q = n // 4
        idx[r0 : r0 + n] = r0 + np.arange(n).reshape(q, 4).T.reshape(-1)
    return idx


_ZOT_IDX = _zot_perm()


def _prep_maps(Z_o, Z_l, Z_g, W, b):
    W32 = np.ascontiguousarray(np.asarray(W, dtype=np.float32))
    b32 = np.ascontiguousarray(np.asarray(b, dtype=np.float32).reshape(CH, 1))
    maps = []
    for i in range(N_CORES):
        sl = slice(i * SHARD, (i + 1) * SHARD)
        zo16 = np.asarray(Z_o[sl], dtype=np.float16)
        zoT = np.ascontiguousarray(zo16[_ZOT_IDX].T)
        maps.append(
            {
                "Z_l": np.ascontiguousarray(np.asarray(Z_l[sl], dtype=np.float16)),
                "Z_g": np.ascontiguousarray(np.asarray(Z_g[sl], dtype=np.float16)),
                "ZoT": zoT,
                "W": W32,
                "b": b32,
            }
        )
    return maps


def kernel(Z_o, Z_l, Z_g, W, b):
    if "nc" not in _CACHE:
        _CACHE["nc"] = build_nc()
    nc = _CACHE["nc"]
    maps = _prep_maps(Z_o, Z_l, Z_g, W, b)
    res = run_bass_kernel_spmd(nc, maps, core_ids=list(range(N_CORES)))
    out = np.concatenate([r["out"] for r in res.results], axis=0)
    return out.astype(np.float32)


# revision 19
# speedup vs baseline: 1.1544x; 1.0247x over previous
"""Trainium2 Bass kernel for nn_Attention2 (attention-gated blend of Z_l/Z_g).

Reference math:
    Q     = Z_o @ W.T + b                      # [N, 512]
    att_l = Q @ colsum(Z_l)                    # [N]
    att_g = Q @ colsum(Z_g)                    # [N]
    att   = softmax([att_l, att_g], axis=1)    # [N, 2]
    out   = Z_l * att[:, 0:1] + Z_g * att[:, 1:2]

Only d = att_l - att_g matters (2-way softmax == sigmoid), and it folds:
    s = colsum(Z_l) - colsum(Z_g)              # [512]
    u = W.T @ s                                # [512]
    c = b . s                                  # scalar
    d = Z_o @ u + c                            # [N]
    out = Z_g + sigmoid(d) * (Z_l - Z_g)

Fused single-launch SPMD design (8 cores, rows sharded, fp16 I/O).

Row packing: each 512-row chunk maps partition q <- rows r0+4q..r0+4q+3
(4KB contiguous per partition = 1 DMA descriptor; DMA issue cost scales with
descriptor count). The shard tail is 212 rows = 4 x 53, so every chunk is
uniform with Q = 128 or 53 partitions. Z_o is uploaded transposed with
columns permuted per chunk so position s*Q + q <-> row r0+4q+s and stage-2
matmul weights slice contiguously.

  Stage 1: stream Z_l (SyncE issue) / Z_g (ScalarE issue) one chunk per DMA;
    colsum via +-1 ones matmuls into PSUM (4 col-slices per chunk); cache
    zd = Z_l - Z_g (all chunks, one DVE sub per chunk) and Z_g (most chunks).
  AllReduce(add) of s_partial [1,512] f32 (DRAM bounce); the first 3 ZoT
    chunk loads are emitted right after stage 1 so their transfers fill the
    collective bubble.
  Interlude: u = W.T @ s (W preloaded in one merged DMA during stage 1);
    u split into fp16 (hi, lo) pair to keep f32 accuracy; c = b.s broadcast.
  Stage 2 per chunk: 32 matmuls (4 s-slices x 4 k-chunks x hi/lo) accumulate
    d in PSUM [Q,1] per s-slice; p = sigmoid(d + c) on ScalarE (reads PSUM);
    out-slice = (zd * p) + zg in one DVE scalar_tensor_tensor; one merged
    out DMA per chunk (SyncE issue).

Precision: fp16 inputs give rel err ~8e-3 vs the f32 reference (gate 2e-2).
"""

import numpy as np

import concourse.bacc as bacc
import concourse.mybir as mybir
import concourse.tile as tile
from concourse.bass_types import AP
from concourse.bass_utils import run_bass_kernel_spmd

N_CORES = 8
N_TOTAL = 100000
CH = 512
SHARD = N_TOTAL // N_CORES  # 12500
P = 128
CHUNK = 4 * P  # 512 rows per chunk
N_CHUNKS = (SHARD + CHUNK - 1) // CHUNK  # 25; last chunk 212 rows (53 x 4)
C_ZG = 12  # Z_g chunks cached in SBUF; rest re-read in stage 2
ZOT_PRE = 3  # zot ring depth / prefetch distance

f16 = mybir.dt.float16
f32 = mybir.dt.float32


def _chunk_rows_ap(dram_t, r0, q):
    """[q, 4*CH] view: partition j <- DRAM rows r0+4j..r0+4j+3 (contiguous)."""
    h = dram_t[0:1].tensor
    return AP(h, r0 * CH, [[4 * CH, q], [1, 4 * CH]])


ZOT_ROW = 4 * SHARD  # packed ZoT: [128, 4*SHARD], chunk ci at col ci*2048


def _zot_ap(dram_t, ci, gw4):
    """[P, gw4] view of packed ZoT: one contiguous run per partition."""
    h = dram_t[0:1].tensor
    return AP(h, ci * (4 * CHUNK), [[ZOT_ROW, P], [1, gw4]])


def _emit_body(nc, pools, tensors, c_zg, no_cc=False, ring3=False):
    add = mybir.AluOpType.add
    mult = mybir.AluOpType.mult
    AF = mybir.ActivationFunctionType
    fix, s1, zop, otp, smp, psfix, psd, dram = pools
    (zl_d, zg_d, zoT_d, w_d, b_d, out_d, consts, zd_cache, zg_cache) = tensors
    ones, nones, one11_16, ones_row, one11_32 = consts

    def chunk_q(ci):
        return min(P, (SHARD - ci * CHUNK + 3) // 4)

    # ---------------- Stage 1: colsum + cache fill ----------------
    # W preload (no deps; lands during stage 1)
    wq = fix.tile([P, 4 * CH], f32, tag="wq")
    h_w = w_d[0:1].tensor
    nc.sync.dma_start(wq[:, :], AP(h_w, 0, [[CH, P], [P * CH, 4], [1, CH]]))
    ps_s = psfix.tile([1, CH], f32, tag="ps_s")
    for ci in range(N_CHUNKS):
        r0 = ci * CHUNK
        Q = chunk_q(ci)
        zl4 = s1.tile([P, 4 * CH], f16, tag="zl4", bufs=2)
        nc.sync.dma_start(zl4[:Q, :], _chunk_rows_ap(zl_d, r0, Q))
        if ci < c_zg:
            zg4, zgc0 = zg_cache, ci * 4 * CH
        else:
            zg4 = s1.tile([P, 4 * CH], f16, tag="zgr", bufs=2, name="zg4t")
            zgc0 = 0
        zg_eng = nc.gpsimd if ring3 else nc.scalar
        zg_eng.dma_start(
            zg4[:Q, zgc0 : zgc0 + 4 * CH], _chunk_rows_ap(zg_d, r0, Q)
        )
        for s in range(4):
            nc.tensor.matmul(
                ps_s[:], ones[:Q], zl4[:Q, s * CH : (s + 1) * CH],
                start=(ci == 0 and s == 0), stop=False,
            )
            nc.tensor.matmul(
                ps_s[:], nones[:Q], zg4[:Q, zgc0 + s * CH : zgc0 + (s + 1) * CH],
                start=False, stop=(ci == N_CHUNKS - 1 and s == 3),
            )
        nc.vector.tensor_sub(
            zd_cache[:Q, ci * 4 * CH : (ci + 1) * 4 * CH],
            zl4[:Q, :],
            zg4[:Q, zgc0 : zgc0 + 4 * CH],
        )

    # ---------------- AllReduce of s ----------------
    s_sb = fix.tile([1, CH], f32, tag="s_sb")
    nc.vector.tensor_copy(s_sb[:], ps_s[:])
    s_part = dram.tile([1, CH], f32, tag="s_part", bufs=2)
    s_glob = dram.tile([1, CH], f32, tag="s_glob", bufs=2)
    nc.sync.dma_start(s_part[:, :], s_sb[:])
    # zot prefetch for the first chunks: no dep on the collective, so these
    # transfers fill the collective bubble.
    zots = {}
    for ci in range(min(ZOT_PRE, N_CHUNKS)):
        zt = zop.tile([P, 4 * CHUNK], f16, tag="zot", bufs=ZOT_PRE, name="zott")
        gw4 = chunk_q(ci) * 16
        nc.scalar.dma_start(zt[:, :gw4], _zot_ap(zoT_d, ci, gw4))
        zots[ci] = zt
    if no_cc:
        nc.sync.dma_start(s_glob[:, :], s_part[:, :])
    else:
        nc.gpsimd.collective_compute(
            "AllReduce",
            add,
            replica_groups=[list(range(N_CORES))],
            ins=[s_part.opt()],
            outs=[s_glob.opt()],
        )
    s_all = fix.tile([1, CH], f32, tag="s_all")
    nc.sync.dma_start(s_all[:], s_glob[:, :])

    # ---------------- Interlude: u, c on device ----------------
    ps_u = psfix.tile([1, CH], f32, tag="ps_u")
    ps_c = psfix.tile([1, 1], f32, tag="ps_c")
    scks = []
    for k in range(4):
        ps_sc = psd.tile([P, 1], f32, tag="tr", bufs=2)
        nc.tensor.matmul(
            ps_sc[:], s_all[0:1, k * P : (k + 1) * P], one11_32[:],
            start=True, stop=True,
        )
        sck = fix.tile([P, 1], f32, tag=f"sck{k}")
        nc.vector.tensor_copy(sck[:], ps_sc[:])
        scks.append(sck)
    for k in range(4):
        nc.tensor.matmul(
            ps_u[:], scks[k][:], wq[:, k * CH : (k + 1) * CH],
            start=(k == 0), stop=(k == 3),
        )
    for k in range(4):
        bk = fix.tile([P, 1], f32, tag=f"bk{k}")
        nc.sync.dma_start(bk[:], b_d[k * P : (k + 1) * P, 0:1])
        nc.tensor.matmul(ps_c[:], scks[k][:], bk[:], start=(k == 0), stop=(k == 3))
    c_sb = fix.tile([1, 1], f32, tag="c_sb")
    nc.vector.tensor_copy(c_sb[:], ps_c[:])
    ps_cb = psd.tile([P, 1], f32, tag="tr", bufs=2)
    nc.tensor.matmul(ps_cb[:], ones_row[:], c_sb[:], start=True, stop=True)
    c_b = fix.tile([P, 1], f32, tag="c_b")
    nc.vector.tensor_copy(c_b[:], ps_cb[:])

    u_hi = fix.tile([1, CH], f16, tag="u_hi")
    nc.vector.tensor_copy(u_hi[:], ps_u[:])
    u_hi32 = fix.tile([1, CH], f32, tag="u_hi32")
    nc.vector.tensor_copy(u_hi32[:], u_hi[:])
    u_lo = fix.tile([1, CH], f16, tag="u_lo")
    nc.vector.tensor_sub(u_lo[:], ps_u[:], u_hi32[:])
    u2 = []
    for k in range(4):
        u2k = fix.tile([P, 2], f16, tag=f"u2_{k}")
        for h, src in enumerate((u_hi, u_lo)):
            ps_tr = psd.tile([P, 1], f32, tag="tr", bufs=2)
            nc.tensor.matmul(
                ps_tr[:], src[0:1, k * P : (k + 1) * P], one11_16[:],
                start=True, stop=True,
            )
            nc.vector.tensor_copy(u2k[:, h : h + 1], ps_tr[:])
        u2.append(u2k)

    # ---------------- Stage 2: d, sigmoid, blend ----------------
    for ci in range(N_CHUNKS):
        r0 = ci * CHUNK
        Q = chunk_q(ci)
        GW = 4 * Q
        zot = zots.pop(ci)
        pre = ci + ZOT_PRE
        if pre < N_CHUNKS:
            zt = zop.tile([P, 4 * CHUNK], f16, tag="zot", bufs=ZOT_PRE, name="zotl")
            gw4p = chunk_q(pre) * 16
            nc.scalar.dma_start(zt[:, :gw4p], _zot_ap(zoT_d, pre, gw4p))
            zots[pre] = zt
        if ci < c_zg:
            zgb, zgc0 = zg_cache, ci * 4 * CH
        else:
            zgb = s1.tile([P, 4 * CH], f16, tag="zgr", bufs=2, name="zgrt")
            zgc0 = 0
            (nc.gpsimd if ring3 else nc.sync).dma_start(
                zgb[:Q, :], _chunk_rows_ap(zg_d, r0, Q)
            )
        outm = otp.tile([P, 4 * CH], f16, tag="outm", bufs=2)
        for s in range(4):
            ps_d = psd.tile([P, 1], f32, tag="d", bufs=3)
            for k in range(4):
                lhs = zot[:, k * GW + s * Q : k * GW + (s + 1) * Q]
                nc.tensor.matmul(
                    ps_d[:Q], lhs, u2[k][:, 0:1], start=(k == 0), stop=False
                )
                nc.tensor.matmul(
                    ps_d[:Q], lhs, u2[k][:, 1:2], start=False, stop=(k == 3)
                )
            p_t = smp.tile([P, 1], f32, tag="p")
            nc.scalar.activation(
                p_t[:Q], ps_d[:Q], AF.Sigmoid, bias=c_b[:Q, 0:1], scale=1.0
            )
            nc.vector.scalar_tensor_tensor(
                outm[:Q, s * CH : (s + 1) * CH],
                zd_cache[:Q, ci * 4 * CH + s * CH : ci * 4 * CH + (s + 1) * CH],
                p_t[:Q, 0:1],
                zgb[:Q, zgc0 + s * CH : zgc0 + (s + 1) * CH],
                op0=mult,
                op1=add,
            )
        out_eng = (nc.scalar if (ring3 and ci % 2) else nc.sync)
        out_eng.dma_start(_chunk_rows_ap(out_d, r0, Q), outm[:Q, :])


def build_nc(c_zg=C_ZG, bufs=4, rep_loop=1, rep_mode="unroll", no_cc=False, ring3=False):
    import contextlib

    nc = bacc.Bacc(
        "TRN2",
        target_bir_lowering=False,
        debug=False,
        enable_asserts=False,
        num_devices=N_CORES,
    )
    zl_d = nc.dram_tensor("Z_l", [SHARD, CH], f16, kind="ExternalInput")
    zg_d = nc.dram_tensor("Z_g", [SHARD, CH], f16, kind="ExternalInput")
    zoT_d = nc.dram_tensor("ZoT", [P, 4 * SHARD], f16, kind="ExternalInput")
    w_d = nc.dram_tensor("W", [CH, CH], f32, kind="ExternalInput")
    b_d = nc.dram_tensor("b", [CH, 1], f32, kind="ExternalInput")
    out_d = nc.dram_tensor("out", [SHARD, CH], f16, kind="ExternalOutput")

    with tile.TileContext(nc) as tc:
        with (
            tc.tile_pool(name="cache", bufs=1) as cache,
            tc.tile_pool(name="fix", bufs=1) as fix,
            tc.tile_pool(name="s1", bufs=2) as s1,
            tc.tile_pool(name="zo", bufs=2) as zop,
            tc.tile_pool(name="ot", bufs=2) as otp,
            tc.tile_pool(name="sm", bufs=4) as smp,
            tc.tile_pool(name="psfix", bufs=1, space="PSUM") as psfix,
            tc.tile_pool(name="psd", bufs=4, space="PSUM") as psd,
            tc.tile_pool(name="dram", bufs=1, space="DRAM") as dram,
        ):
            ones = fix.tile([P, 1], f16, tag="ones")
            nones = fix.tile([P, 1], f16, tag="nones")
            one11_16 = fix.tile([1, 1], f16, tag="one11_16")
            ones_row = fix.tile([1, P], f32, tag="ones_row")
            one11_32 = fix.tile([1, 1], f32, tag="one11_32")
            nc.vector.memset(ones[:], 1.0)
            nc.vector.memset(nones[:], -1.0)
            nc.vector.memset(one11_16[:], 1.0)
            nc.vector.memset(ones_row[:], 1.0)
            nc.vector.memset(one11_32[:], 1.0)
            consts = (ones, nones, one11_16, ones_row, one11_32)

            zd_cache = cache.tile([P, N_CHUNKS * 4 * CH], f16, tag="zd")
            zg_cache = cache.tile([P, c_zg * 4 * CH], f16, tag="zg")

            pools = (fix, s1, zop, otp, smp, psfix, psd, dram)
            tensors = (
                zl_d, zg_d, zoT_d, w_d, b_d, out_d, consts, zd_cache, zg_cache
            )
            if rep_loop > 1 and rep_mode == "unroll":
                for _ in range(rep_loop):
                    _emit_body(nc, pools, tensors, c_zg, no_cc=no_cc, ring3=ring3)
            else:
                rep_ctx = (
                    tc.For_i(0, rep_loop, 1)
                    if rep_loop > 1
                    else contextlib.nullcontext()
                )
                with rep_ctx:
                    _emit_body(nc, pools, tensors, c_zg, no_cc=no_cc, ring3=ring3)
    nc.compile()
    return nc


_CACHE = {}


# column permutation applied to zoT within each 512-row chunk:
# position s*Q + q  <->  row r0 + 4q + s
def _zot_perm():
    idx = np.empty(SHARD, dtype=np.int64)
    for ci in range(N_CHUNKS):
        r0 = ci * CHUNK
        n = min(CHUNK, SHARD - r0)
        q = n // 4
        idx[r0 : r0 + n] = r0 + np.arange(n).reshape(q, 4).T.reshape(-1)
    return idx


_ZOT_IDX = _zot_perm()


def _prep_maps(Z_o, Z_l, Z_g, W, b):
    W32 = np.ascontiguousarray(np.asarray(W, dtype=np.float32))
    b32 = np.ascontiguousarray(np.asarray(b, dtype=np.float32).reshape(CH, 1))
    maps = []
    for i in range(N_CORES):
        sl = slice(i * SHARD, (i + 1) * SHARD)
        zo16 = np.asarray(Z_o[sl], dtype=np.float16)
        zoTf = zo16[_ZOT_IDX].T  # [512, SHARD], columns chunk-permuted
        full = np.transpose(
            zoTf[:, : 24 * CHUNK].reshape(4, P, 24, CHUNK), (1, 2, 0, 3)
        ).reshape(P, -1)
        tail = np.transpose(
            zoTf[:, 24 * CHUNK :].reshape(4, P, SHARD - 24 * CHUNK), (1, 0, 2)
        ).reshape(P, -1)
        zoT = np.ascontiguousarray(np.concatenate([full, tail], axis=1))
        maps.append(
            {
                "Z_l": np.ascontiguousarray(np.asarray(Z_l[sl], dtype=np.float16)),
                "Z_g": np.ascontiguousarray(np.asarray(Z_g[sl], dtype=np.float16)),
                "ZoT": zoT,
                "W": W32,
                "b": b32,
            }
        )
    return maps


def kernel(Z_o, Z_l, Z_g, W, b):
    if "nc" not in _CACHE:
        _CACHE["nc"] = build_nc()
    nc = _CACHE["nc"]
    maps = _prep_maps(Z_o, Z_l, Z_g, W, b)
    res = run_bass_kernel_spmd(nc, maps, core_ids=list(range(N_CORES)))
    out = np.concatenate([r["out"] for r in res.results], axis=0)
    return out.astype(np.float32)


# revision 21
# speedup vs baseline: 1.4407x; 1.2480x over previous
"""Trainium2 Bass kernel for nn_Attention2 (attention-gated blend of Z_l/Z_g).

Reference math:
    Q     = Z_o @ W.T + b                      # [N, 512]
    att_l = Q @ colsum(Z_l)                    # [N]
    att_g = Q @ colsum(Z_g)                    # [N]
    att   = softmax([att_l, att_g], axis=1)    # [N, 2]
    out   = Z_l * att[:, 0:1] + Z_g * att[:, 1:2]

Only d = att_l - att_g matters (2-way softmax == sigmoid), and it folds:
    s = colsum(Z_l) - colsum(Z_g)              # [512]
    u = W.T @ s                                # [512]
    c = b . s                                  # scalar
    d = Z_o @ u + c                            # [N]
    out = Z_g + sigmoid(d) * (Z_l - Z_g)

Fused single-launch SPMD design (8 cores, rows sharded, fp16 I/O).

Row packing: each 512-row chunk maps partition q <- rows r0+4q..r0+4q+3
(4KB contiguous per partition = 1 DMA descriptor; DMA issue cost scales with
descriptor count). The shard tail is 212 rows = 4 x 53, so every chunk is
uniform with Q = 128 or 53 partitions. Z_o is uploaded transposed with
columns permuted per chunk so position s*Q + q <-> row r0+4q+s and stage-2
matmul weights slice contiguously.

  Stage 1: stream Z_l (SyncE issue) / Z_g (ScalarE issue) one chunk per DMA;
    colsum via +-1 ones matmuls into PSUM (4 col-slices per chunk); cache
    zd = Z_l - Z_g (all chunks, one DVE sub per chunk) and Z_g (most chunks).
  AllReduce(add) of s_partial [1,512] f32 (DRAM bounce); the first 3 ZoT
    chunk loads are emitted right after stage 1 so their transfers fill the
    collective bubble.
  Interlude: u = W.T @ s (W preloaded in one merged DMA during stage 1);
    u split into fp16 (hi, lo) pair to keep f32 accuracy; c = b.s broadcast.
  Stage 2 per chunk: 32 matmuls (4 s-slices x 4 k-chunks x hi/lo) accumulate
    d in PSUM [Q,1] per s-slice; p = sigmoid(d + c) on ScalarE (reads PSUM);
    out-slice = (zd * p) + zg in one DVE scalar_tensor_tensor; one merged
    out DMA per chunk (SyncE issue).

Precision: fp16 inputs give rel err ~8e-3 vs the f32 reference (gate 2e-2).
"""

import numpy as np

import concourse.bacc as bacc
import concourse.mybir as mybir
import concourse.tile as tile
from concourse.bass_types import AP
from concourse.bass_utils import run_bass_kernel_spmd

N_CORES = 8
N_TOTAL = 100000
CH = 512
SHARD = N_TOTAL // N_CORES  # 12500
P = 128
CHUNK = 4 * P  # 512 rows per chunk
N_CHUNKS = (SHARD + CHUNK - 1) // CHUNK  # 25; last chunk 212 rows (53 x 4)
C_ZG = 12  # Z_g chunks cached in SBUF; rest re-read in stage 2
ZOT_PRE = 3  # zot ring depth / prefetch distance

f16 = mybir.dt.float16
f32 = mybir.dt.float32


def _chunk_rows_ap(dram_t, r0, q):
    """[q, 4*CH] view: partition j <- DRAM rows r0+4j..r0+4j+3 (contiguous)."""
    h = dram_t[0:1].tensor
    return AP(h, r0 * CH, [[4 * CH, q], [1, 4 * CH]])


ZOT_ROW = 4 * SHARD  # packed ZoT: [128, 4*SHARD], chunk ci at col ci*2048


def _zot_ap(dram_t, ci, gw4):
    """[P, gw4] view of packed ZoT: one contiguous run per partition."""
    h = dram_t[0:1].tensor
    return AP(h, ci * (4 * CHUNK), [[ZOT_ROW, P], [1, gw4]])


def _emit_body(nc, pools, tensors, c_zg, no_cc=False, ring3=False):
    add = mybir.AluOpType.add
    mult = mybir.AluOpType.mult
    AF = mybir.ActivationFunctionType
    fix, s1, zop, otp, smp, psfix, psd, dram = pools
    (zl_d, zg_d, zoT_d, w_d, b_d, out_d, consts, zd_cache, zg_cache) = tensors
    ones, nones, one11_16, ones_row, one11_32 = consts

    def chunk_q(ci):
        return min(P, (SHARD - ci * CHUNK + 3) // 4)

    # ---------------- Stage 1: colsum + cache fill ----------------
    # W preload (no deps; lands during stage 1)
    wq = fix.tile([P, 4 * CH], f32, tag="wq")
    h_w = w_d[0:1].tensor
    nc.sync.dma_start(wq[:, :], AP(h_w, 0, [[CH, P], [P * CH, 4], [1, CH]]))
    ps_s = psfix.tile([1, CH], f32, tag="ps_s")
    for ci in range(N_CHUNKS):
        r0 = ci * CHUNK
        Q = chunk_q(ci)
        zl4 = s1.tile([P, 4 * CH], f16, tag="zl4", bufs=2)
        nc.sync.dma_start(zl4[:Q, :], _chunk_rows_ap(zl_d, r0, Q))
        if ci < c_zg:
            zg4, zgc0 = zg_cache, ci * 4 * CH
        else:
            zg4 = s1.tile([P, 4 * CH], f16, tag="zgr", bufs=2, name="zg4t")
            zgc0 = 0
        zg_eng = nc.gpsimd if ring3 else nc.scalar
        zg_eng.dma_start(
            zg4[:Q, zgc0 : zgc0 + 4 * CH], _chunk_rows_ap(zg_d, r0, Q)
        )
        for s in range(4):
            nc.tensor.matmul(
                ps_s[:], ones[:Q], zl4[:Q, s * CH : (s + 1) * CH],
                start=(ci == 0 and s == 0), stop=False,
            )
            nc.tensor.matmul(
                ps_s[:], nones[:Q], zg4[:Q, zgc0 + s * CH : zgc0 + (s + 1) * CH],
                start=False, stop=(ci == N_CHUNKS - 1 and s == 3),
            )
        nc.vector.tensor_sub(
            zd_cache[:Q, ci * 4 * CH : (ci + 1) * 4 * CH],
            zl4[:Q, :],
            zg4[:Q, zgc0 : zgc0 + 4 * CH],
        )

    # ---------------- AllReduce of s ----------------
    s_sb = fix.tile([1, CH], f32, tag="s_sb")
    nc.vector.tensor_copy(s_sb[:], ps_s[:])
    s_part = dram.tile([1, CH], f32, tag="s_part", bufs=2)
    s_glob = dram.tile([1, CH], f32, tag="s_glob", bufs=2)
    nc.sync.dma_start(s_part[:, :], s_sb[:])
    # zot prefetch for the first chunks: no dep on the collective, so these
    # transfers fill the collective bubble.
    zots = {}
    for ci in range(min(ZOT_PRE, N_CHUNKS)):
        zt = zop.tile([P, 4 * CHUNK], f16, tag="zot", bufs=ZOT_PRE, name="zott")
        gw4 = chunk_q(ci) * 16
        nc.scalar.dma_start(zt[:, :gw4], _zot_ap(zoT_d, ci, gw4))
        zots[ci] = zt
    if no_cc:
        nc.sync.dma_start(s_glob[:, :], s_part[:, :])
    else:
        nc.gpsimd.collective_compute(
            "AllReduce",
            add,
            replica_groups=[list(range(N_CORES))],
            ins=[s_part.opt()],
            outs=[s_glob.opt()],
        )
    s_all = fix.tile([1, CH], f32, tag="s_all")
    nc.sync.dma_start(s_all[:], s_glob[:, :])

    # ---------------- Interlude: u, c on device ----------------
    ps_u = psfix.tile([1, CH], f32, tag="ps_u")
    ps_c = psfix.tile([1, 1], f32, tag="ps_c")
    scks = []
    for k in range(4):
        ps_sc = psd.tile([P, 1], f32, tag="tr", bufs=2)
        nc.tensor.matmul(
            ps_sc[:], s_all[0:1, k * P : (k + 1) * P], one11_32[:],
            start=True, stop=True,
        )
        sck = fix.tile([P, 1], f32, tag=f"sck{k}")
        nc.vector.tensor_copy(sck[:], ps_sc[:])
        scks.append(sck)
    for k in range(4):
        nc.tensor.matmul(
            ps_u[:], scks[k][:], wq[:, k * CH : (k + 1) * CH],
            start=(k == 0), stop=(k == 3),
        )
    for k in range(4):
        bk = fix.tile([P, 1], f32, tag=f"bk{k}")
        nc.sync.dma_start(bk[:], b_d[k * P : (k + 1) * P, 0:1])
        nc.tensor.matmul(ps_c[:], scks[k][:], bk[:], start=(k == 0), stop=(k == 3))
    c_sb = fix.tile([1, 1], f32, tag="c_sb")
    nc.vector.tensor_copy(c_sb[:], ps_c[:])
    ps_cb = psd.tile([P, 1], f32, tag="tr", bufs=2)
    nc.tensor.matmul(ps_cb[:], ones_row[:], c_sb[:], start=True, stop=True)
    c_b = fix.tile([P, 1], f32, tag="c_b")
    nc.vector.tensor_copy(c_b[:], ps_cb[:])

    u_hi = fix.tile([1, CH], f16, tag="u_hi")
    nc.vector.tensor_copy(u_hi[:], ps_u[:])
    u_hi32 = fix.tile([1, CH], f32, tag="u_hi32")
    nc.vector.tensor_copy(u_hi32[:], u_hi[:])
    u_lo = fix.tile([1, CH], f16, tag="u_lo")
    nc.vector.tensor_sub(u_lo[:], ps_u[:], u_hi32[:])
    u2 = []
    for k in range(4):
        u2k = fix.tile([P, 2], f16, tag=f"u2_{k}")
        for h, src in enumerate((u_hi, u_lo)):
            ps_tr = psd.tile([P, 1], f32, tag="tr", bufs=2)
            nc.tensor.matmul(
                ps_tr[:], src[0:1, k * P : (k + 1) * P], one11_16[:],
                start=True, stop=True,
            )
            nc.vector.tensor_copy(u2k[:, h : h + 1], ps_tr[:])
        u2.append(u2k)

    # ---------------- Stage 2: d, sigmoid, blend ----------------
    for ci in range(N_CHUNKS):
        r0 = ci * CHUNK
        Q = chunk_q(ci)
        GW = 4 * Q
        zot = zots.pop(ci)
        pre = ci + ZOT_PRE
        if pre < N_CHUNKS:
            zt = zop.tile([P, 4 * CHUNK], f16, tag="zot", bufs=ZOT_PRE, name="zotl")
            gw4p = chunk_q(pre) * 16
            nc.scalar.dma_start(zt[:, :gw4p], _zot_ap(zoT_d, pre, gw4p))
            zots[pre] = zt
        if ci < c_zg:
            zgb, zgc0 = zg_cache, ci * 4 * CH
        else:
            zgb = s1.tile([P, 4 * CH], f16, tag="zgr", bufs=2, name="zgrt")
            zgc0 = 0
            (nc.gpsimd if ring3 else nc.sync).dma_start(
                zgb[:Q, :], _chunk_rows_ap(zg_d, r0, Q)
            )
        outm = otp.tile([P, 4 * CH], f16, tag="outm", bufs=2)
        for s in range(4):
            ps_d = psd.tile([P, 1], f32, tag="d", bufs=3)
            for k in range(4):
                lhs = zot[:, k * GW + s * Q : k * GW + (s + 1) * Q]
                nc.tensor.matmul(
                    ps_d[:Q], lhs, u2[k][:, 0:1], start=(k == 0), stop=False
                )
                nc.tensor.matmul(
                    ps_d[:Q], lhs, u2[k][:, 1:2], start=False, stop=(k == 3)
                )
            p_t = smp.tile([P, 1], f32, tag="p")
            nc.scalar.activation(
                p_t[:Q], ps_d[:Q], AF.Sigmoid, bias=c_b[:Q, 0:1], scale=1.0
            )
            nc.vector.scalar_tensor_tensor(
                outm[:Q, s * CH : (s + 1) * CH],
                zd_cache[:Q, ci * 4 * CH + s * CH : ci * 4 * CH + (s + 1) * CH],
                p_t[:Q, 0:1],
                zgb[:Q, zgc0 + s * CH : zgc0 + (s + 1) * CH],
                op0=mult,
                op1=add,
            )
        out_eng = (nc.scalar if (ring3 and ci % 2) else nc.sync)
        out_eng.dma_start(_chunk_rows_ap(out_d, r0, Q), outm[:Q, :])


def build_nc(c_zg=C_ZG, bufs=4, rep_loop=1, rep_mode="unroll", no_cc=False, ring3=False):
    import contextlib

    nc = bacc.Bacc(
        "TRN2",
        target_bir_lowering=False,
        debug=False,
        enable_asserts=False,
        num_devices=N_CORES,
    )
    zl_d = nc.dram_tensor("Z_l", [SHARD, CH], f16, kind="ExternalInput")
    zg_d = nc.dram_tensor("Z_g", [SHARD, CH], f16, kind="ExternalInput")
    zoT_d = nc.dram_tensor("ZoT", [P, 4 * SHARD], f16, kind="ExternalInput")
    w_d = nc.dram_tensor("W", [CH, CH], f32, kind="ExternalInput")
    b_d = nc.dram_tensor("b", [CH, 1], f32, kind="ExternalInput")
    out_d = nc.dram_tensor("out", [SHARD, CH], f16, kind="ExternalOutput")

    with tile.TileContext(nc) as tc:
        with (
            tc.tile_pool(name="cache", bufs=1) as cache,
            tc.tile_pool(name="fix", bufs=1) as fix,
            tc.tile_pool(name="s1", bufs=2) as s1,
            tc.tile_pool(name="zo", bufs=2) as zop,
            tc.tile_pool(name="ot", bufs=2) as otp,
            tc.tile_pool(name="sm", bufs=4) as smp,
            tc.tile_pool(name="psfix", bufs=1, space="PSUM") as psfix,
            tc.tile_pool(name="psd", bufs=4, space="PSUM") as psd,
            tc.tile_pool(name="dram", bufs=1, space="DRAM") as dram,
        ):
            ones = fix.tile([P, 1], f16, tag="ones")
            nones = fix.tile([P, 1], f16, tag="nones")
            one11_16 = fix.tile([1, 1], f16, tag="one11_16")
            ones_row = fix.tile([1, P], f32, tag="ones_row")
            one11_32 = fix.tile([1, 1], f32, tag="one11_32")
            nc.vector.memset(ones[:], 1.0)
            nc.vector.memset(nones[:], -1.0)
            nc.vector.memset(one11_16[:], 1.0)
            nc.vector.memset(ones_row[:], 1.0)
            nc.vector.memset(one11_32[:], 1.0)
            consts = (ones, nones, one11_16, ones_row, one11_32)

            zd_cache = cache.tile([P, N_CHUNKS * 4 * CH], f16, tag="zd")
            zg_cache = cache.tile([P, c_zg * 4 * CH], f16, tag="zg")

            pools = (fix, s1, zop, otp, smp, psfix, psd, dram)
            tensors = (
                zl_d, zg_d, zoT_d, w_d, b_d, out_d, consts, zd_cache, zg_cache
            )
            if rep_loop > 1 and rep_mode == "unroll":
                for _ in range(rep_loop):
                    _emit_body(nc, pools, tensors, c_zg, no_cc=no_cc, ring3=ring3)
            else:
                rep_ctx = (
                    tc.For_i(0, rep_loop, 1)
                    if rep_loop > 1
                    else contextlib.nullcontext()
                )
                with rep_ctx:
                    _emit_body(nc, pools, tensors, c_zg, no_cc=no_cc, ring3=ring3)
    nc.compile()
    return nc


_CACHE = {}


# column permutation applied to zoT within each 512-row chunk:
# position s*Q + q  <->  row r0 + 4q + s
def _zot_perm():
    idx = np.empty(SHARD, dtype=np.int64)
    for ci in range(N_CHUNKS):
        r0 = ci * CHUNK
        n = min(CHUNK, SHARD - r0)
        q = n // 4
        idx[r0 : r0 + n] = r0 + np.arange(n).reshape(q, 4).T.reshape(-1)
    return idx


_ZOT_IDX = _zot_perm()


def _prep_maps(Z_o, Z_l, Z_g, W, b):
    W32 = np.ascontiguousarray(np.asarray(W, dtype=np.float32))
    b32 = np.ascontiguousarray(np.asarray(b, dtype=np.float32).reshape(CH, 1))
    maps = []
    for i in range(N_CORES):
        sl = slice(i * SHARD, (i + 1) * SHARD)
        zo16 = np.asarray(Z_o[sl], dtype=np.float16)
        zoTf = zo16[_ZOT_IDX].T  # [512, SHARD], columns chunk-permuted
        full = np.transpose(
            zoTf[:, : 24 * CHUNK].reshape(4, P, 24, CHUNK), (1, 2, 0, 3)
        ).reshape(P, -1)
        tail = np.transpose(
            zoTf[:, 24 * CHUNK :].reshape(4, P, SHARD - 24 * CHUNK), (1, 0, 2)
        ).reshape(P, -1)
        zoT = np.ascontiguousarray(np.concatenate([full, tail], axis=1))
        maps.append(
            {
                "Z_l": np.ascontiguousarray(np.asarray(Z_l[sl], dtype=np.float16)),
                "Z_g": np.ascontiguousarray(np.asarray(Z_g[sl], dtype=np.float16)),
                "ZoT": zoT,
                "W": W32,
                "b": b32,
            }
        )
    return maps


def kernel(Z_o, Z_l, Z_g, W, b):
    if "nc" not in _CACHE:
        _CACHE["nc"] = build_nc()
    nc = _CACHE["nc"]
    maps = _prep_maps(Z_o, Z_l, Z_g, W, b)
    res = run_bass_kernel_spmd(nc, maps, core_ids=list(range(N_CORES)))
    out = np.concatenate([r["out"] for r in res.results], axis=0)
    return out.astype(np.float32)
